# revision 19
# baseline (speedup 1.0000x reference)
"""Trainium2 Bass kernel for GQA attention prefill (Llama-style).

Reference computation (fp32):
  xq = x@wq.T+bq; xk = x@wk.T+bk; xv = x@wv.T+bv
  rope(xq, xk); scores = q@k.T/sqrt(128) + causal_mask
  probs = softmax(scores); out = (probs@v) reshaped @ wo.T + bo

Shapes: x [2, 2048, 4096], 32 q heads / 8 kv heads, head_dim 128.

Sharding: TP=4 over head groups (8 q heads + 2 kv heads per group) x DP=2
over batch -> 8 cores. Each core computes a partial output [2048, 4096]
(its heads' contribution through wo columns); host sums the 4 partials
per batch. Biases bq/bk applied on device (ACT bias); bv/bo folded into a
constant host-side row (sum_k probs == 1).

On-device dataflow (all matmuls bf16 with fp32 PSUM accumulation):
  - host passes xT [4096, 2048] bf16 so projections produce q/k in
    [head_dim, tok] layout directly (no transposes anywhere).
  - RoPE with de-interleaved head_dim (host permutes wq/wk rows per head:
    evens then odds); rotation = swap 64-partition halves via SBUF->SBUF
    DMA; cos/sin staged as [128, 2048] with sign folded into sin.
  - phase 2 processes HEAD PAIRS sharing one kv head: the two heads'
    S_T / exp / PV interleave so each exp's latency hides behind the
    partner head's matmuls, and K/V weight loads are shared.
  - scores computed transposed S_T[k, q]; softmax denominators for both
    heads accumulate into ONE PSUM bank (partitions 0 / 32) via grouped
    ones-matmul bursts emitted after each 4-kt block (no exp-wait).
  - causal mask = binary multiply on the 4 diagonal-block offsets only;
    blocks fully above the diagonal are skipped.
  - PV output lands in [head_dim, tok] layout == lhsT of the wo matmul.
  - phase 3 keeps a 4-wide oc quad of wo resident so each attn lhsT load
    feeds 4 consecutive matmuls; wo streams in on the idle SP ring
    during phase 2.
"""
import sys

for _p in ("/opt/trn_rl_repo",):
    if _p not in sys.path:
        sys.path.insert(0, _p)

from contextlib import ExitStack

import ml_dtypes
import numpy as np

import concourse.bass as bass  # noqa: F401  (AP types used implicitly)
import concourse.tile as tile
from concourse import bacc, mybir
from concourse import masks as masks_mod
from concourse.bass_utils import run_bass_kernel_spmd

BF16 = ml_dtypes.bfloat16
F32 = np.float32

DIM = 4096
SEQ = 2048
HD = 128
N_HEADS = 32
N_KV = 8
QH = 8          # q heads per core
KVH = 2         # kv heads per core
NM = KVH + KVH + QH   # 12 m-tiles per core: V0 V1 K0 K1 Q0..Q7
TC = 512        # token chunk (PSUM free dim)
NKK = DIM // 128      # 32 contraction chunks
NTCH = SEQ // TC      # 4 token chunks
NTT = SEQ // 128      # 16 token tiles
SCALE = 1.0 / float(np.sqrt(HD))

# de-interleave permutation within one head_dim: evens then odds
PERM = np.concatenate([np.arange(0, HD, 2), np.arange(1, HD, 2)])

_CACHE = {}
LAST_RESULT = None


def _emit(tc_ctx, nc, xT, wT, woT, cosd, sind, mbin, bqd, bkd, out, phases=3,
          attn_mode="full", depth=3, sgrp=4, out_bf16=False):
    dt = mybir.dt
    AF = mybir.ActivationFunctionType
    with ExitStack() as ctx:
        cpool = ctx.enter_context(tc_ctx.tile_pool(name="consts", bufs=1))
        xpool = ctx.enter_context(tc_ctx.tile_pool(name="xp", bufs=10))
        wpool = ctx.enter_context(tc_ctx.tile_pool(name="wp", bufs=10))
        wopool = ctx.enter_context(tc_ctx.tile_pool(name="wop", bufs=1))
        qkv = ctx.enter_context(tc_ctx.tile_pool(name="qkv", bufs=1))
        stage = ctx.enter_context(tc_ctx.tile_pool(name="stage", bufs=2))
        probs_p = ctx.enter_context(tc_ctx.tile_pool(name="probs", bufs=4))
        aupool = ctx.enter_context(tc_ctx.tile_pool(name="aup", bufs=2))
        rpool = ctx.enter_context(tc_ctx.tile_pool(name="rp", bufs=2))
        pvsp = ctx.enter_context(tc_ctx.tile_pool(name="pvs", bufs=2))
        outp = ctx.enter_context(tc_ctx.tile_pool(name="outp", bufs=1))
        # PSUM as three double-bank-oriented pools:
        #   ps_w: 3 slots of [128, 1024] fp32 (6 banks) shared by all phases
        #   ps_a: 1 bank, phase-2 PV accumulator
        #   ps_sum: 1 bank, phase-2 softmax denominators
        ps_w = ctx.enter_context(tc_ctx.tile_pool(name="psw", bufs=3, space="PSUM"))
        ps_a = ctx.enter_context(tc_ctx.tile_pool(name="psa", bufs=1, space="PSUM"))
        ps_sum = ctx.enter_context(tc_ctx.tile_pool(name="pssum", bufs=1, space="PSUM"))

        # ---- constant tiles (DMAs emitted after the first x/w loads so the
        # first matmuls aren't queued behind 4.5 MiB of constants) ----
        cos_sb = cpool.tile([HD, SEQ], dt.bfloat16, name="cos_sb")
        sin_sb = cpool.tile([HD, SEQ], dt.bfloat16, name="sin_sb")
        mask_sb = cpool.tile([HD, 4 * TC], dt.bfloat16, name="mask_sb")
        bq_sb = cpool.tile([HD, QH], dt.float32, name="bq_sb")
        bk_sb = cpool.tile([HD, KVH], dt.float32, name="bk_sb")
        ones_col = cpool.tile([128, 1], dt.bfloat16, name="ones_col")
        ident = cpool.tile([128, 128], dt.bfloat16, name="ident")

        def emit_const_loads():
            nc.sync.dma_start(cos_sb[:], cosd[:])
            nc.sync.dma_start(sin_sb[:], sind[:])
            nc.sync.dma_start(mask_sb[:], mbin[:])
            nc.sync.dma_start(bq_sb[:], bqd[:])
            nc.sync.dma_start(bk_sb[:], bkd[:])
            nc.vector.memset(ones_col[:], 1.0)
            masks_mod.make_identity(nc, ident[:])

        # ---- persistent per-core tensors ----
        q_roped = [qkv.tile([HD, SEQ], dt.bfloat16, name=f"qr{h}", tag=f"qr{h}")
                   for h in range(QH)]
        k_roped = [qkv.tile([HD, SEQ], dt.bfloat16, name=f"kr{g}", tag=f"kr{g}")
                   for g in range(KVH)]
        v_sb = [qkv.tile([128, KVH * HD], dt.bfloat16, name=f"vs{t}", tag=f"vs{t}")
                for t in range(NTT)]
        attn_out = [qkv.tile([HD, SEQ], dt.bfloat16, name=f"ao{h}", tag=f"ao{h}")
                    for h in range(QH)]

        # rope is emitted in three batched stages so no engine's in-order
        # queue chains one rope's tail into the next rope's PSUM release:
        #   stage 1 (ACT/DVE alternating): bias-add, PSUM -> SBUF, frees bank
        #   stage 2 (Pool): 64-partition half swap via SBUF->SBUF DMA
        #   stage 3 (DVE): cos/sin muls + add into the roped destination
        def rope_s1(pst, bias_ap, j, on_act):
            pre = stage.tile([128, TC], dt.bfloat16, name=f"pre{j}",
                             tag=f"pre{j}", bufs=1)
            if on_act:
                nc.scalar.activation(pre[:], pst[:], AF.Identity, bias=bias_ap)
            else:
                nc.vector.tensor_scalar_add(pre[:], pst[:], bias_ap)
            return pre

        def rope_s2(pre, j):
            rot = stage.tile([128, TC], dt.bfloat16, name=f"rot{j}",
                             tag=f"rot{j}", bufs=1)
            nc.gpsimd.dma_start(rot[0:64, :], pre[64:128, :])
            nc.gpsimd.dma_start(rot[64:128, :], pre[0:64, :])
            return rot

        def rope_s3(pre, rot, dst, t):
            sl = slice(t * TC, (t + 1) * TC)
            nc.vector.tensor_mul(pre[:], pre[:], cos_sb[:, sl])
            nc.vector.tensor_mul(rot[:], rot[:], sin_sb[:, sl])
            nc.vector.tensor_add(dst[:, sl], pre[:], rot[:])

        # ---- phase 1 (kk-outer streaming): per tok chunk, two m-groups of
        # 6 PSUM accumulators; each x / weight tile is consumed by 6 back-to-
        # back matmuls and freed, so DMA streams continuously. The V
        # transposes for a chunk are emitted AFTER group 1's matmuls so the
        # PE never waits on the group-0 epilogue drain.
        GRP = 6
        for t in range(NTCH):
            vtmp = [None, None]
            for grp in range(2):
                mb = grp * GRP
                slots = [ps_w.tile([128, 2 * TC], dt.float32,
                                   name=f"pw{sl}", tag="w")
                         for sl in range(GRP // 2)]
                pss = [slots[j // 2][:, (j % 2) * TC:(j % 2 + 1) * TC]
                       for j in range(GRP)]
                for kk in range(NKK):
                    # alternate the two HWDGE rings (sync / scalar) so x and
                    # weight streams run on parallel queues
                    ex = nc.sync if kk % 2 == 0 else nc.scalar
                    ew = nc.scalar if kk % 2 == 0 else nc.sync
                    xt = xpool.tile([128, TC], dt.bfloat16, name="xt", tag="xt")
                    ex.dma_start(
                        xt[:], xT[kk * 128:(kk + 1) * 128, t * TC:(t + 1) * TC])
                    wk6 = wpool.tile([128, GRP * 128], dt.bfloat16, name="wk6",
                                     tag="wk6")
                    ew.dma_start(
                        wk6[:],
                        wT[kk * 128:(kk + 1) * 128, mb * 128:(mb + GRP) * 128])
                    if t == 0 and grp == 0 and kk == 0:
                        emit_const_loads()

                    for j in range(GRP):
                        nc.tensor.matmul(pss[j][:],
                                         wk6[:, j * 128:(j + 1) * 128], xt[:],
                                         start=(kk == 0), stop=(kk == NKK - 1),
                                         skip_group_check=True)

                # group epilogue, batched by stage. Stage 1 is emitted at
                # high priority so the scheduler keeps it ahead of the next
                # group's DMA triggers on the ACT/DVE queues (the PSUM ring
                # release gates the next group's matmuls).
                ropes = []
                with tc_ctx.high_priority():
                    for j in range(GRP):
                        m = mb + j
                        on_act = (j % 2 == 0)
                        if m < KVH:  # V heads: keep [d, tok], no rope
                            vt_g = stage.tile([128, TC], dt.bfloat16,
                                              name=f"vtmp{m}", tag=f"vd{m}")
                            if on_act:
                                nc.scalar.copy(vt_g[:], pss[j][:])
                            else:
                                nc.vector.tensor_copy(vt_g[:], pss[j][:])
                            vtmp[m] = vt_g
                        elif m < 2 * KVH:  # K heads
                            g = m - KVH
                            pre = rope_s1(pss[j], bk_sb[:, g:g + 1], j, on_act)
                            ropes.append((pre, k_roped[g]))
                        else:  # Q heads
                            h = m - 2 * KVH
                            pre = rope_s1(pss[j], bq_sb[:, h:h + 1], j, on_act)
                            ropes.append((pre, q_roped[h]))
                rots = [rope_s2(pre, j) for j, (pre, _) in enumerate(ropes)]
                for (pre, dst), rot in zip(ropes, rots):
                    rope_s3(pre, rot, dst, t)
            # V transpose to [tok, d]; emitted after BOTH groups so the PE
            # pipeline isn't blocked on the group-0 epilogue. PSUM from the
            # "pv" ring (unused in phase 1).
            vslot = ps_a.tile([128, 8 * 128], dt.bfloat16, name="vtr",
                              tag="pv")
            for i4 in range(4):
                tt = t * 4 + i4
                for g in range(KVH):
                    i8 = i4 * 2 + g
                    pvt = vslot[:, i8 * 128:(i8 + 1) * 128]
                    nc.tensor.transpose(
                        pvt, vtmp[g][:, i4 * 128:(i4 + 1) * 128],
                        ident[:])
                    if i8 % 2 == 0:
                        nc.scalar.copy(v_sb[tt][:, g * 128:(g + 1) * 128],
                                       pvt)
                    else:
                        nc.vector.tensor_copy(
                            v_sb[tt][:, g * 128:(g + 1) * 128], pvt)

        if phases <= 1:
            # pin phase-1 outputs so DCE keeps the work
            for h in range(QH):
                src = q_roped[h][:, 0:256]
                nc.sync.dma_start(out[h * 128:(h + 1) * 128, 0:256] if out_bf16
                                  else out[h * 128:(h + 1) * 128, 0:128],
                                  src if out_bf16 else src.bitcast(dt.float32))
            for g in range(KVH):
                src = k_roped[g][:, 0:256]
                nc.sync.dma_start(
                    out[1024 + g * 128:1024 + (g + 1) * 128, 0:256] if out_bf16
                    else out[1024 + g * 128:1024 + (g + 1) * 128, 0:128],
                    src if out_bf16 else src.bitcast(dt.float32))
            for tt in range(NTT):
                nc.sync.dma_start(out[tt * 128:(tt + 1) * 128, 512:768] if out_bf16
                                  else out[tt * 128:(tt + 1) * 128, 512:640],
                                  v_sb[tt][:, :] if out_bf16
                                  else v_sb[tt][:, :].bitcast(dt.float32))
            return

        # ---- phase 2: attention per head, kt-PAIRS in double-bank PSUM ----
        # Each ST slot holds scores for two k-tiles ([128, 2*TC] fp32, two
        # banks). Off-diagonal pairs get ONE wide exp over both banks --
        # fewer ACT instructions. Diagonal pairs keep per-half exps with
        # the column trim + mask. PV is copied to SBUF right at stop so
        # its 1-bank ring never gates the next iteration. Consecutive
        # (h, qc) iterations are software-pipelined: the next iteration's
        # first two ST pairs are emitted BEFORE the previous iteration's
        # final denominator burst + tail, so the exp latency of the new
        # pipeline's head hides behind real PE work.
        def p2_st_pair(p, g, qc, h):
            slot = ps_w.tile([128, 2 * TC], dt.float32, name="stw", tag="w")
            os_ = []
            for half in range(2):
                kt = 2 * p + half
                o = (kt - 4 * qc) * 128 if kt >= 4 * qc else 0
                nc.tensor.matmul(
                    slot[:, half * TC + o:(half + 1) * TC],
                    k_roped[g][:, kt * 128:(kt + 1) * 128],
                    q_roped[h][:, qc * TC + o:(qc + 1) * TC],
                    start=True, stop=True)
                os_.append(o)
            return slot, os_

        def p2_head(h, qc):
            g = h // (QH // KVH)
            nkt = 4 * qc + 4
            npair = nkt // 2
            st = dict(h=h, qc=qc, g=g, nkt=nkt, npair=npair,
                      pend={}, edone={})
            st["sums3"] = ps_sum.tile([1, TC], dt.float32, name="sums3",
                                      tag="s3")
            st["pv"] = ps_a.tile([128, TC], dt.float32, name="pv", tag="pv")
            st["pend"][0] = p2_st_pair(0, g, qc, h)
            if npair > 1:
                st["pend"][1] = p2_st_pair(1, g, qc, h)
            return st

        def p2_burst(st, ps_):
            # denominator burst: consecutive ones-matmuls, no exp-wait
            for p in ps_:
                e, os_ = st["edone"].pop(p)
                for half in range(2):
                    kt = 2 * p + half
                    o = os_[half]
                    nc.tensor.matmul(
                        st["sums3"][0:1, o:TC], ones_col[:],
                        e[:, half * TC + o:(half + 1) * TC],
                        start=(kt == 0), stop=(kt == st["nkt"] - 1),
                        skip_group_check=True)

        def p2_main(st):
            h, qc, g = st["h"], st["qc"], st["g"]
            nkt, npair = st["nkt"], st["npair"]
            last_p0 = ((npair - 1) // 2) * 2
            for p0 in range(0, npair, 2):
                ps_ = list(range(p0, min(p0 + 2, npair)))
                for p in ps_:
                    if p + 2 < npair:
                        st["pend"][p + 2] = p2_st_pair(p + 2, g, qc, h)
                    slot, os_ = st["pend"].pop(p)
                    e = probs_p.tile([128, 2 * TC], dt.bfloat16,
                                     name="e", tag="e")
                    diag = (2 * p + 1) >= 4 * qc
                    if not diag:
                        nc.scalar.activation(e[:], slot[:], AF.Exp,
                                             scale=SCALE)
                    else:
                        for half in range(2):
                            kt = 2 * p + half
                            o = os_[half]
                            sl_ = slice(half * TC + o, (half + 1) * TC)
                            nc.scalar.activation(e[:, sl_], slot[:, sl_],
                                                 AF.Exp, scale=SCALE)
                            if attn_mode != "nomask" and kt >= 4 * qc:
                                oi = kt - 4 * qc
                                nc.vector.tensor_mul(
                                    e[:, sl_], e[:, sl_],
                                    mask_sb[:, oi * TC + o:(oi + 1) * TC])
                    st["edone"][p] = (e, os_)
                    for half in range(2):
                        kt = 2 * p + half
                        o = os_[half]
                        nc.tensor.matmul(
                            st["pv"][:, o:TC],
                            v_sb[kt][:, g * HD:(g + 1) * HD],
                            e[:, half * TC + o:(half + 1) * TC],
                            start=(kt == 0), stop=(kt == nkt - 1),
                            skip_group_check=True)
                if attn_mode not in ("nosums", "nonorm") and p0 != last_p0:
                    p2_burst(st, ps_)
            st["last_ps"] = list(range(last_p0, npair))

        def p2_tail(st):
            h, qc = st["h"], st["qc"]
            if attn_mode in ("nosums", "nonorm"):
                st["edone"].clear()
                nc.vector.tensor_copy(attn_out[h][:, qc * TC:(qc + 1) * TC],
                                      st["pv"][:])
                return
            p2_burst(st, st["last_ps"])
            # move PV off PSUM immediately so the 1-slot ring frees
            pvs = pvsp.tile([128, TC], dt.float32, name="pvs", tag="pvs")
            nc.vector.tensor_copy(pvs[:], st["pv"][:])
            recip = rpool.tile([1, TC], dt.float32, name="r", tag="r")
            nc.vector.reciprocal(recip[:], st["sums3"][0:1, :])
            bcs = aupool.tile([128, TC], dt.float32, name="bcs", tag="b")
            nc.gpsimd.partition_broadcast(bcs[:], recip[:])
            nc.vector.tensor_mul(attn_out[h][:, qc * TC:(qc + 1) * TC],
                                 pvs[:], bcs[:])

        prev = None
        for h in range(QH):
            for qc in range(NTCH):
                st = p2_head(h, qc)
                if prev is not None:
                    p2_tail(prev)
                p2_main(st)
                prev = st
        p2_tail(prev)

        if phases <= 2:
            for h in range(QH):
                src = attn_out[h][:, 0:2048]
                nc.sync.dma_start(out[h * 128:(h + 1) * 128, 0:2048] if out_bf16
                                  else out[h * 128:(h + 1) * 128, 2048:3072],
                                  src if out_bf16 else src.bitcast(dt.float32))
            return

        # ---- phase 3: output projection (partial over this core's heads).
        # 4-wide oc quads of wo stay resident (loaded on the SP ring during
        # phase 2); each attn lhsT load feeds 4 consecutive matmuls.
        NQ = 4
        for q4 in range(DIM // TC // NQ):
            wos = []
            for h2 in range(QH):
                row = []
                for oc2 in range(NQ):
                    wo_t = wopool.tile([128, TC], dt.bfloat16,
                                       name=f"wo{h2}_{oc2}",
                                       tag=f"wo{h2}_{oc2}")
                    oc = q4 * NQ + oc2
                    nc.sync.dma_start(
                        wo_t[:],
                        woT[h2 * 128:(h2 + 1) * 128, oc * TC:(oc + 1) * TC])
                    row.append(wo_t)
                wos.append(row)
            for tt in range(NTT):
                poslots = [ps_w.tile([128, 2 * TC], dt.float32,
                                     name=f"pow{i}", tag="w")
                           for i in range(NQ // 2)]
                pos = [poslots[oc2 // 2][:, (oc2 % 2) * TC:(oc2 % 2 + 1) * TC]
                       for oc2 in range(NQ)]
                for h2 in range(QH):
                    at = attn_out[h2][:, tt * 128:(tt + 1) * 128]
                    for oc2 in range(NQ):
                        nc.tensor.matmul(pos[oc2][:], at, wos[h2][oc2][:],
                                         start=(h2 == 0), stop=(h2 == QH - 1),
                                         skip_group_check=True)
                for oc2 in range(NQ):
                    oc = q4 * NQ + oc2
                    ob = outp.tile([128, TC],
                                   dt.bfloat16 if out_bf16 else dt.float32,
                                   name="ob", tag="ob", bufs=3)
                    if oc2 % 2 == 0:
                        nc.vector.tensor_copy(ob[:], pos[oc2][:])
                    else:
                        nc.scalar.copy(ob[:], pos[oc2][:])
                    nc.sync.dma_start(
                        out[tt * 128:(tt + 1) * 128, oc * TC:(oc + 1) * TC],
                        ob[:])


def build_nc(num_devices=8, reps=1, phases=3, attn_mode="full", depth=3,
             sgrp=4, out_bf16=False):
    nc = bacc.Bacc("TRN2", target_bir_lowering=False, debug=False,
                   enable_asserts=False, num_devices=num_devices)
    dt = mybir.dt
    xT = nc.dram_tensor("xT", [DIM, SEQ], dt.bfloat16, kind="ExternalInput").ap()
    wT = nc.dram_tensor("wT", [DIM, NM * 128], dt.bfloat16,
                        kind="ExternalInput").ap()
    woT = nc.dram_tensor("woT", [QH * HD, DIM], dt.bfloat16,
                         kind="ExternalInput").ap()
    cosd = nc.dram_tensor("cos128", [HD, SEQ], dt.bfloat16,
                          kind="ExternalInput").ap()
    sind = nc.dram_tensor("sin128s", [HD, SEQ], dt.bfloat16,
                          kind="ExternalInput").ap()
    mbin = nc.dram_tensor("maskbin", [HD, 4 * TC], dt.bfloat16,
                          kind="ExternalInput").ap()
    bqd = nc.dram_tensor("bq_sb", [HD, QH], dt.float32,
                         kind="ExternalInput").ap()
    bkd = nc.dram_tensor("bk_sb", [HD, KVH], dt.float32,
                         kind="ExternalInput").ap()
    out = nc.dram_tensor("out", [SEQ, DIM],
                         dt.bfloat16 if out_bf16 else dt.float32,
                         kind="ExternalOutput").ap()
    kw = dict(phases=phases, attn_mode=attn_mode, depth=depth, sgrp=sgrp,
              out_bf16=out_bf16)
    with tile.TileContext(nc) as tctx:
        if reps == 1:
            _emit(tctx, nc, xT, wT, woT, cosd, sind, mbin, bqd, bkd, out, **kw)
        else:
            with tctx.For_i(0, reps, 1):
                _emit(tctx, nc, xT, wT, woT, cosd, sind, mbin, bqd, bkd, out,
                      **kw)
    nc.compile()
    return nc


def _get_nc():
    if "nc" not in _CACHE:
        _CACHE["nc"] = build_nc()
    return _CACHE["nc"]


def make_in_maps(x, freqs_cos, freqs_sin, mask, wq, bq, wk, bk, wv, bv):
    x = np.asarray(x, F32)
    xT_b = [np.ascontiguousarray(x[b].T).astype(BF16) for b in range(x.shape[0])]
    cosT = np.asarray(freqs_cos, F32).T  # [64, 2048]
    sinT = np.asarray(freqs_sin, F32).T
    cos128 = np.ascontiguousarray(np.vstack([cosT, cosT])).astype(BF16)
    sin128s = np.ascontiguousarray(np.vstack([-sinT, sinT])).astype(BF16)
    mask = np.asarray(mask, F32)
    mbin = np.zeros((HD, 4 * TC), F32)
    for oi in range(4):
        blk = mask[3 * TC:4 * TC, (12 + oi) * 128:(13 + oi) * 128]  # [512 q, 128 k]
        mbin[:, oi * TC:(oi + 1) * TC] = (blk == 0).T
    mbin = np.ascontiguousarray(mbin).astype(BF16)

    wq, wk, wv = (np.asarray(a, F32) for a in (wq, wk, wv))
    bq, bk = np.asarray(bq, F32), np.asarray(bk, F32)
    group_maps = []
    for g in range(4):
        wq_g = wq[g * 1024:(g + 1) * 1024].reshape(QH, HD, DIM)[:, PERM, :]
        wk_g = wk[g * 256:(g + 1) * 256].reshape(KVH, HD, DIM)[:, PERM, :]
        wv_g = wv[g * 256:(g + 1) * 256]
        wcat = np.concatenate(
            [wv_g, wk_g.reshape(KVH * HD, DIM), wq_g.reshape(QH * HD, DIM)], axis=0)
        wT_g = np.ascontiguousarray(wcat.T).astype(BF16)  # [4096, 1536]
        bq_g = np.ascontiguousarray(
            bq[g * 1024:(g + 1) * 1024].reshape(QH, HD)[:, PERM].T).astype(F32)
        bk_g = np.ascontiguousarray(
            bk[g * 256:(g + 1) * 256].reshape(KVH, HD)[:, PERM].T).astype(F32)
        group_maps.append(dict(wT=wT_g, bq_sb=bq_g, bk_sb=bk_g))

    in_maps = []
    for c in range(8):
        b, g = c // 4, c % 4
        in_maps.append(dict(xT=xT_b[b], cos128=cos128, sin128s=sin128s,
                            maskbin=mbin, **group_maps[g]))
    return in_maps


def kernel(x, freqs_cos, freqs_sin, mask, wq, bq, wk, bk, wv, bv, wo, bo):
    global LAST_RESULT
    nc = _get_nc()
    wo = np.asarray(wo, F32)
    in_maps = make_in_maps(x, freqs_cos, freqs_sin, mask, wq, bq, wk, bk, wv, bv)
    for c in range(8):
        g = c % 4
        in_maps[c]["woT"] = np.ascontiguousarray(
            wo[:, g * 1024:(g + 1) * 1024].T).astype(BF16)
    res = run_bass_kernel_spmd(nc, in_maps, core_ids=list(range(8)))
    LAST_RESULT = res
    outp = np.zeros((2, SEQ, DIM), F32)
    for c in range(8):
        outp[c // 4] += np.asarray(res.results[c]["out"], F32)
    bv = np.asarray(bv, F32)
    bo = np.asarray(bo, F32)
    bv_exp = np.broadcast_to(
        bv.reshape(N_KV, 1, HD), (N_KV, N_HEADS // N_KV, HD)).reshape(DIM)
    outp += (bv_exp @ wo.T + bo)[None, None, :].astype(F32)
    return outp


# revision 20
# speedup vs baseline: 1.0129x; 1.0129x over previous
"""Trainium2 Bass kernel for GQA attention prefill (Llama-style).

Reference computation (fp32):
  xq = x@wq.T+bq; xk = x@wk.T+bk; xv = x@wv.T+bv
  rope(xq, xk); scores = q@k.T/sqrt(128) + causal_mask
  probs = softmax(scores); out = (probs@v) reshaped @ wo.T + bo

Shapes: x [2, 2048, 4096], 32 q heads / 8 kv heads, head_dim 128.

Sharding: TP=4 over head groups (8 q heads + 2 kv heads per group) x DP=2
over batch -> 8 cores. Each core computes a partial output [2048, 4096]
(its heads' contribution through wo columns); host sums the 4 partials
per batch. Biases bq/bk applied on device (ACT bias); bv/bo folded into a
constant host-side row (sum_k probs == 1).

On-device dataflow (all matmuls bf16 with fp32 PSUM accumulation):
  - host passes xT [4096, 2048] bf16 so projections produce q/k in
    [head_dim, tok] layout directly (no transposes anywhere).
  - RoPE with de-interleaved head_dim (host permutes wq/wk rows per head:
    evens then odds); rotation = swap 64-partition halves via SBUF->SBUF
    DMA; cos/sin staged as [128, 2048] with sign folded into sin.
  - phase 2 processes HEAD PAIRS sharing one kv head: the two heads'
    S_T / exp / PV interleave so each exp's latency hides behind the
    partner head's matmuls, and K/V weight loads are shared.
  - scores computed transposed S_T[k, q]; softmax denominators for both
    heads accumulate into ONE PSUM bank (partitions 0 / 32) via grouped
    ones-matmul bursts emitted after each 4-kt block (no exp-wait).
  - causal mask = binary multiply on the 4 diagonal-block offsets only;
    blocks fully above the diagonal are skipped.
  - PV output lands in [head_dim, tok] layout == lhsT of the wo matmul.
  - phase 3 keeps a 4-wide oc quad of wo resident so each attn lhsT load
    feeds 4 consecutive matmuls; wo streams in on the idle SP ring
    during phase 2.
"""
import sys

for _p in ("/opt/trn_rl_repo",):
    if _p not in sys.path:
        sys.path.insert(0, _p)

from contextlib import ExitStack

import ml_dtypes
import numpy as np

import concourse.bass as bass  # noqa: F401  (AP types used implicitly)
import concourse.tile as tile
from concourse import bacc, mybir
from concourse import masks as masks_mod
from concourse.bass_utils import run_bass_kernel_spmd

BF16 = ml_dtypes.bfloat16
F32 = np.float32

DIM = 4096
SEQ = 2048
HD = 128
N_HEADS = 32
N_KV = 8
QH = 8          # q heads per core
KVH = 2         # kv heads per core
NM = KVH + KVH + QH   # 12 m-tiles per core: V0 V1 K0 K1 Q0..Q7
TC = 512        # token chunk (PSUM free dim)
NKK = DIM // 128      # 32 contraction chunks
NTCH = SEQ // TC      # 4 token chunks
NTT = SEQ // 128      # 16 token tiles
SCALE = 1.0 / float(np.sqrt(HD))

# de-interleave permutation within one head_dim: evens then odds
PERM = np.concatenate([np.arange(0, HD, 2), np.arange(1, HD, 2)])

_CACHE = {}
LAST_RESULT = None


def _emit(tc_ctx, nc, xT, wT, woT, cosd, sind, mbin, bqd, bkd, out, phases=3,
          attn_mode="full", depth=3, sgrp=4, out_bf16=False):
    dt = mybir.dt
    AF = mybir.ActivationFunctionType
    with ExitStack() as ctx:
        cpool = ctx.enter_context(tc_ctx.tile_pool(name="consts", bufs=1))
        xpool = ctx.enter_context(tc_ctx.tile_pool(name="xp", bufs=11))
        wpool = ctx.enter_context(tc_ctx.tile_pool(name="wp", bufs=11))
        wopool = ctx.enter_context(tc_ctx.tile_pool(name="wop", bufs=1))
        qkv = ctx.enter_context(tc_ctx.tile_pool(name="qkv", bufs=1))
        stage = ctx.enter_context(tc_ctx.tile_pool(name="stage", bufs=2))
        probs_p = ctx.enter_context(tc_ctx.tile_pool(name="probs", bufs=5))
        aupool = ctx.enter_context(tc_ctx.tile_pool(name="aup", bufs=2))
        rpool = ctx.enter_context(tc_ctx.tile_pool(name="rp", bufs=2))
        outp = ctx.enter_context(tc_ctx.tile_pool(name="outp", bufs=1))
        # PSUM: 6-bank shared ring (tag "a") + 2 banks for PV pairs /
        # phase-1 V transposes (tag "pv")
        ps_a = ctx.enter_context(tc_ctx.tile_pool(name="psa", bufs=6, space="PSUM"))
        ps_sum = ctx.enter_context(tc_ctx.tile_pool(name="pssum", bufs=2, space="PSUM"))

        # ---- constant tiles (DMAs emitted after the first x/w loads so the
        # first matmuls aren't queued behind 4.5 MiB of constants) ----
        cos_sb = cpool.tile([HD, SEQ], dt.bfloat16, name="cos_sb")
        sin_sb = cpool.tile([HD, SEQ], dt.bfloat16, name="sin_sb")
        mask_sb = cpool.tile([HD, 4 * TC], dt.bfloat16, name="mask_sb")
        bq_sb = cpool.tile([HD, QH], dt.float32, name="bq_sb")
        bk_sb = cpool.tile([HD, KVH], dt.float32, name="bk_sb")
        ones_col = cpool.tile([128, 1], dt.bfloat16, name="ones_col")
        ident = cpool.tile([128, 128], dt.bfloat16, name="ident")

        def emit_const_loads():
            nc.sync.dma_start(cos_sb[:], cosd[:])
            nc.sync.dma_start(sin_sb[:], sind[:])
            nc.sync.dma_start(mask_sb[:], mbin[:])
            nc.sync.dma_start(bq_sb[:], bqd[:])
            nc.sync.dma_start(bk_sb[:], bkd[:])
            nc.vector.memset(ones_col[:], 1.0)
            masks_mod.make_identity(nc, ident[:])

        # ---- persistent per-core tensors ----
        q_roped = [qkv.tile([HD, SEQ], dt.bfloat16, name=f"qr{h}", tag=f"qr{h}")
                   for h in range(QH)]
        k_roped = [qkv.tile([HD, SEQ], dt.bfloat16, name=f"kr{g}", tag=f"kr{g}")
                   for g in range(KVH)]
        v_sb = [qkv.tile([128, KVH * HD], dt.bfloat16, name=f"vs{t}", tag=f"vs{t}")
                for t in range(NTT)]
        attn_out = [qkv.tile([HD, SEQ], dt.bfloat16, name=f"ao{h}", tag=f"ao{h}")
                    for h in range(QH)]

        # rope is emitted in three batched stages so no engine's in-order
        # queue chains one rope's tail into the next rope's PSUM release:
        #   stage 1 (ACT/DVE alternating): bias-add, PSUM -> SBUF, frees bank
        #   stage 2 (Pool): 64-partition half swap via SBUF->SBUF DMA
        #   stage 3 (DVE): cos/sin muls + add into the roped destination
        def rope_s1(pst, bias_ap, j, on_act):
            pre = stage.tile([128, TC], dt.bfloat16, name=f"pre{j}",
                             tag=f"pre{j}", bufs=1)
            if on_act:
                nc.scalar.activation(pre[:], pst[:], AF.Identity, bias=bias_ap)
            else:
                nc.vector.tensor_scalar_add(pre[:], pst[:], bias_ap)
            return pre

        def rope_s2(pre, j):
            rot = stage.tile([128, TC], dt.bfloat16, name=f"rot{j}",
                             tag=f"rot{j}", bufs=1)
            nc.gpsimd.dma_start(rot[0:64, :], pre[64:128, :])
            nc.gpsimd.dma_start(rot[64:128, :], pre[0:64, :])
            return rot

        def rope_s3(pre, rot, dst, t):
            sl = slice(t * TC, (t + 1) * TC)
            nc.vector.tensor_mul(pre[:], pre[:], cos_sb[:, sl])
            nc.vector.tensor_mul(rot[:], rot[:], sin_sb[:, sl])
            nc.vector.tensor_add(dst[:, sl], pre[:], rot[:])

        # ---- phase 1 (kk-outer streaming): per tok chunk, two m-groups of
        # 6 PSUM accumulators; each x / weight tile is consumed by 6 back-to-
        # back matmuls and freed, so DMA streams continuously. The V
        # transposes for a chunk are emitted AFTER group 1's matmuls so the
        # PE never waits on the group-0 epilogue drain.
        GRP = 6
        for t in range(NTCH):
            vtmp = [None, None]
            for grp in range(2):
                mb = grp * GRP
                pss = [ps_a.tile([128, TC], dt.float32, name=f"pj{j}", tag="a")
                       for j in range(GRP)]
                for kk in range(NKK):
                    # alternate the two HWDGE rings (sync / scalar) so x and
                    # weight streams run on parallel queues
                    ex = nc.sync if kk % 2 == 0 else nc.scalar
                    ew = nc.scalar if kk % 2 == 0 else nc.sync
                    xt = xpool.tile([128, TC], dt.bfloat16, name="xt", tag="xt")
                    ex.dma_start(
                        xt[:], xT[kk * 128:(kk + 1) * 128, t * TC:(t + 1) * TC])
                    wk6 = wpool.tile([128, GRP * 128], dt.bfloat16, name="wk6",
                                     tag="wk6")
                    ew.dma_start(
                        wk6[:],
                        wT[kk * 128:(kk + 1) * 128, mb * 128:(mb + GRP) * 128])
                    if t == 0 and grp == 0 and kk == 0:
                        emit_const_loads()

                    for j in range(GRP):
                        nc.tensor.matmul(pss[j][:],
                                         wk6[:, j * 128:(j + 1) * 128], xt[:],
                                         start=(kk == 0), stop=(kk == NKK - 1),
                                         skip_group_check=True)

                # group epilogue, batched by stage. Stage 1 is emitted at
                # high priority so the scheduler keeps it ahead of the next
                # group's DMA triggers on the ACT/DVE queues (the PSUM ring
                # release gates the next group's matmuls).
                ropes = []
                with tc_ctx.high_priority():
                    for j in range(GRP):
                        m = mb + j
                        on_act = (j % 2 == 0)
                        if m < KVH:  # V heads: keep [d, tok], no rope
                            vt_g = stage.tile([128, TC], dt.bfloat16,
                                              name=f"vtmp{m}", tag=f"vd{m}")
                            if on_act:
                                nc.scalar.copy(vt_g[:], pss[j][:])
                            else:
                                nc.vector.tensor_copy(vt_g[:], pss[j][:])
                            vtmp[m] = vt_g
                        elif m < 2 * KVH:  # K heads
                            g = m - KVH
                            pre = rope_s1(pss[j], bk_sb[:, g:g + 1], j, on_act)
                            ropes.append((pre, k_roped[g]))
                        else:  # Q heads
                            h = m - 2 * KVH
                            pre = rope_s1(pss[j], bq_sb[:, h:h + 1], j, on_act)
                            ropes.append((pre, q_roped[h]))
                rots = [rope_s2(pre, j) for j, (pre, _) in enumerate(ropes)]
                for (pre, dst), rot in zip(ropes, rots):
                    rope_s3(pre, rot, dst, t)
            # V transpose to [tok, d]; emitted after BOTH groups so the PE
            # pipeline isn't blocked on the group-0 epilogue. PSUM from the
            # "pv" ring (unused in phase 1).
            for i4 in range(4):
                tt = t * 4 + i4
                for g in range(KVH):
                    pvt = ps_sum.tile([128, 128], dt.bfloat16,
                                      name="pvt", tag="ps2")
                    nc.tensor.transpose(
                        pvt[:], vtmp[g][:, i4 * 128:(i4 + 1) * 128],
                        ident[:])
                    if (i4 + g) % 2 == 0:
                        nc.scalar.copy(v_sb[tt][:, g * 128:(g + 1) * 128],
                                       pvt[:])
                    else:
                        nc.vector.tensor_copy(
                            v_sb[tt][:, g * 128:(g + 1) * 128], pvt[:])

        if phases <= 1:
            # pin phase-1 outputs so DCE keeps the work
            for h in range(QH):
                src = q_roped[h][:, 0:256]
                nc.sync.dma_start(out[h * 128:(h + 1) * 128, 0:256] if out_bf16
                                  else out[h * 128:(h + 1) * 128, 0:128],
                                  src if out_bf16 else src.bitcast(dt.float32))
            for g in range(KVH):
                src = k_roped[g][:, 0:256]
                nc.sync.dma_start(
                    out[1024 + g * 128:1024 + (g + 1) * 128, 0:256] if out_bf16
                    else out[1024 + g * 128:1024 + (g + 1) * 128, 0:128],
                    src if out_bf16 else src.bitcast(dt.float32))
            for tt in range(NTT):
                nc.sync.dma_start(out[tt * 128:(tt + 1) * 128, 512:768] if out_bf16
                                  else out[tt * 128:(tt + 1) * 128, 512:640],
                                  v_sb[tt][:, :] if out_bf16
                                  else v_sb[tt][:, :].bitcast(dt.float32))
            return

        # ---- phase 2: attention per head (v1 structure: depth-3 ST
        # lookahead in the 6-slot ring, one spare slot so ST allocation
        # never serializes on exp completion) ----
        for h in range(QH):
            g = h // (QH // KVH)
            for qc in range(NTCH):
                nkt = 4 * qc + 4

                def st_of(kt, _h=h, _g=g, _qc=qc):
                    o = (kt - 4 * _qc) * 128 if kt >= 4 * _qc else 0
                    st = ps_a.tile([128, TC], dt.float32, name="st", tag="a")
                    nc.tensor.matmul(
                        st[:, o:TC],
                        k_roped[_g][:, kt * 128:(kt + 1) * 128],
                        q_roped[_h][:, _qc * TC + o:(_qc + 1) * TC],
                        start=True, stop=True)
                    return st, o

                sums = ps_sum.tile([1, TC], dt.float32, name="sums",
                                   tag="ps2")
                pv = ps_a.tile([128, TC], dt.float32, name="pv", tag="a")
                sts = [st_of(k) for k in range(min(depth, nkt))]
                for kt in range(nkt):
                    if kt + depth < nkt:
                        sts.append(st_of(kt + depth))
                    st, o = sts[kt]
                    e = probs_p.tile([128, TC], dt.bfloat16, name="expS",
                                     tag="e0")
                    nc.scalar.activation(e[:, o:TC], st[:, o:TC], AF.Exp,
                                         scale=SCALE)
                    if attn_mode != "nomask" and kt >= 4 * qc:
                        oi = kt - 4 * qc
                        nc.vector.tensor_mul(
                            e[:, o:TC], e[:, o:TC],
                            mask_sb[:, oi * TC + o:(oi + 1) * TC])
                    if attn_mode not in ("nosums", "nonorm"):
                        nc.tensor.matmul(sums[0:1, o:TC], ones_col[:],
                                         e[:, o:TC],
                                         start=(kt == 0), stop=(kt == nkt - 1),
                                         skip_group_check=True)
                    nc.tensor.matmul(pv[:, o:TC],
                                     v_sb[kt][:, g * HD:(g + 1) * HD],
                                     e[:, o:TC], start=(kt == 0),
                                     stop=(kt == nkt - 1),
                                     skip_group_check=True)
                if attn_mode in ("nosums", "nonorm"):
                    nc.vector.tensor_copy(
                        attn_out[h][:, qc * TC:(qc + 1) * TC], pv[:])
                    continue
                recip = rpool.tile([1, TC], dt.float32, name="recip",
                                   tag="r0")
                nc.vector.reciprocal(recip[:], sums[0:1, :])
                bcs = aupool.tile([128, TC], dt.float32, name="bcs", tag="b0")
                nc.gpsimd.partition_broadcast(bcs[:], recip[:])
                nc.vector.tensor_mul(attn_out[h][:, qc * TC:(qc + 1) * TC],
                                     pv[:], bcs[:])

        if phases <= 2:
            for h in range(QH):
                src = attn_out[h][:, 0:2048]
                nc.sync.dma_start(out[h * 128:(h + 1) * 128, 0:2048] if out_bf16
                                  else out[h * 128:(h + 1) * 128, 2048:3072],
                                  src if out_bf16 else src.bitcast(dt.float32))
            return

        # ---- phase 3: output projection (partial over this core's heads).
        # 4-wide oc quads of wo stay resident (loaded on the SP ring during
        # phase 2); each attn lhsT load feeds 4 consecutive matmuls.
        NQ = 4
        for q4 in range(DIM // TC // NQ):
            wos = []
            for h2 in range(QH):
                row = []
                for oc2 in range(NQ):
                    wo_t = wopool.tile([128, TC], dt.bfloat16,
                                       name=f"wo{h2}_{oc2}",
                                       tag=f"wo{h2}_{oc2}")
                    oc = q4 * NQ + oc2
                    nc.sync.dma_start(
                        wo_t[:],
                        woT[h2 * 128:(h2 + 1) * 128, oc * TC:(oc + 1) * TC])
                    row.append(wo_t)
                wos.append(row)
            for tt in range(NTT):
                pos = [ps_a.tile([128, TC], dt.float32, name=f"po{oc2}",
                                 tag="a") for oc2 in range(NQ)]
                for h2 in range(QH):
                    at = attn_out[h2][:, tt * 128:(tt + 1) * 128]
                    for oc2 in range(NQ):
                        nc.tensor.matmul(pos[oc2][:], at, wos[h2][oc2][:],
                                         start=(h2 == 0), stop=(h2 == QH - 1),
                                         skip_group_check=True)
                for oc2 in range(NQ):
                    oc = q4 * NQ + oc2
                    ob = outp.tile([128, TC],
                                   dt.bfloat16 if out_bf16 else dt.float32,
                                   name="ob", tag="ob", bufs=3)
                    if oc2 % 2 == 0:
                        nc.vector.tensor_copy(ob[:], pos[oc2][:])
                    else:
                        nc.scalar.copy(ob[:], pos[oc2][:])
                    nc.sync.dma_start(
                        out[tt * 128:(tt + 1) * 128, oc * TC:(oc + 1) * TC],
                        ob[:])


def build_nc(num_devices=8, reps=1, phases=3, attn_mode="full", depth=3,
             sgrp=4, out_bf16=False):
    nc = bacc.Bacc("TRN2", target_bir_lowering=False, debug=False,
                   enable_asserts=False, num_devices=num_devices)
    dt = mybir.dt
    xT = nc.dram_tensor("xT", [DIM, SEQ], dt.bfloat16, kind="ExternalInput").ap()
    wT = nc.dram_tensor("wT", [DIM, NM * 128], dt.bfloat16,
                        kind="ExternalInput").ap()
    woT = nc.dram_tensor("woT", [QH * HD, DIM], dt.bfloat16,
                         kind="ExternalInput").ap()
    cosd = nc.dram_tensor("cos128", [HD, SEQ], dt.bfloat16,
                          kind="ExternalInput").ap()
    sind = nc.dram_tensor("sin128s", [HD, SEQ], dt.bfloat16,
                          kind="ExternalInput").ap()
    mbin = nc.dram_tensor("maskbin", [HD, 4 * TC], dt.bfloat16,
                          kind="ExternalInput").ap()
    bqd = nc.dram_tensor("bq_sb", [HD, QH], dt.float32,
                         kind="ExternalInput").ap()
    bkd = nc.dram_tensor("bk_sb", [HD, KVH], dt.float32,
                         kind="ExternalInput").ap()
    out = nc.dram_tensor("out", [SEQ, DIM],
                         dt.bfloat16 if out_bf16 else dt.float32,
                         kind="ExternalOutput").ap()
    kw = dict(phases=phases, attn_mode=attn_mode, depth=depth, sgrp=sgrp,
              out_bf16=out_bf16)
    with tile.TileContext(nc) as tctx:
        if reps == 1:
            _emit(tctx, nc, xT, wT, woT, cosd, sind, mbin, bqd, bkd, out, **kw)
        else:
            with tctx.For_i(0, reps, 1):
                _emit(tctx, nc, xT, wT, woT, cosd, sind, mbin, bqd, bkd, out,
                      **kw)
    nc.compile()
    return nc


def _get_nc():
    if "nc" not in _CACHE:
        _CACHE["nc"] = build_nc()
    return _CACHE["nc"]


def make_in_maps(x, freqs_cos, freqs_sin, mask, wq, bq, wk, bk, wv, bv):
    x = np.asarray(x, F32)
    xT_b = [np.ascontiguousarray(x[b].T).astype(BF16) for b in range(x.shape[0])]
    cosT = np.asarray(freqs_cos, F32).T  # [64, 2048]
    sinT = np.asarray(freqs_sin, F32).T
    cos128 = np.ascontiguousarray(np.vstack([cosT, cosT])).astype(BF16)
    sin128s = np.ascontiguousarray(np.vstack([-sinT, sinT])).astype(BF16)
    mask = np.asarray(mask, F32)
    mbin = np.zeros((HD, 4 * TC), F32)
    for oi in range(4):
        blk = mask[3 * TC:4 * TC, (12 + oi) * 128:(13 + oi) * 128]  # [512 q, 128 k]
        mbin[:, oi * TC:(oi + 1) * TC] = (blk == 0).T
    mbin = np.ascontiguousarray(mbin).astype(BF16)

    wq, wk, wv = (np.asarray(a, F32) for a in (wq, wk, wv))
    bq, bk = np.asarray(bq, F32), np.asarray(bk, F32)
    group_maps = []
    for g in range(4):
        wq_g = wq[g * 1024:(g + 1) * 1024].reshape(QH, HD, DIM)[:, PERM, :]
        wk_g = wk[g * 256:(g + 1) * 256].reshape(KVH, HD, DIM)[:, PERM, :]
        wv_g = wv[g * 256:(g + 1) * 256]
        wcat = np.concatenate(
            [wv_g, wk_g.reshape(KVH * HD, DIM), wq_g.reshape(QH * HD, DIM)], axis=0)
        wT_g = np.ascontiguousarray(wcat.T).astype(BF16)  # [4096, 1536]
        bq_g = np.ascontiguousarray(
            bq[g * 1024:(g + 1) * 1024].reshape(QH, HD)[:, PERM].T).astype(F32)
        bk_g = np.ascontiguousarray(
            bk[g * 256:(g + 1) * 256].reshape(KVH, HD)[:, PERM].T).astype(F32)
        group_maps.append(dict(wT=wT_g, bq_sb=bq_g, bk_sb=bk_g))

    in_maps = []
    for c in range(8):
        b, g = c // 4, c % 4
        in_maps.append(dict(xT=xT_b[b], cos128=cos128, sin128s=sin128s,
                            maskbin=mbin, **group_maps[g]))
    return in_maps


def kernel(x, freqs_cos, freqs_sin, mask, wq, bq, wk, bk, wv, bv, wo, bo):
    global LAST_RESULT
    nc = _get_nc()
    wo = np.asarray(wo, F32)
    in_maps = make_in_maps(x, freqs_cos, freqs_sin, mask, wq, bq, wk, bk, wv, bv)
    for c in range(8):
        g = c % 4
        in_maps[c]["woT"] = np.ascontiguousarray(
            wo[:, g * 1024:(g + 1) * 1024].T).astype(BF16)
    res = run_bass_kernel_spmd(nc, in_maps, core_ids=list(range(8)))
    LAST_RESULT = res
    outp = np.zeros((2, SEQ, DIM), F32)
    for c in range(8):
        outp[c // 4] += np.asarray(res.results[c]["out"], F32)
    bv = np.asarray(bv, F32)
    bo = np.asarray(bo, F32)
    bv_exp = np.broadcast_to(
        bv.reshape(N_KV, 1, HD), (N_KV, N_HEADS // N_KV, HD)).reshape(DIM)
    outp += (bv_exp @ wo.T + bo)[None, None, :].astype(F32)
    return outp


# revision 21
# speedup vs baseline: 1.0558x; 1.0423x over previous
"""Trainium2 Bass kernel for GQA attention prefill (Llama-style).

Reference computation (fp32):
  xq = x@wq.T+bq; xk = x@wk.T+bk; xv = x@wv.T+bv
  rope(xq, xk); scores = q@k.T/sqrt(128) + causal_mask
  probs = softmax(scores); out = (probs@v) reshaped @ wo.T + bo

Shapes: x [2, 2048, 4096], 32 q heads / 8 kv heads, head_dim 128.

Sharding: TP=4 over head groups (8 q heads + 2 kv heads per group) x DP=2
over batch -> 8 cores. Each core computes a partial output [2048, 4096]
(its heads' contribution through wo columns); host sums the 4 partials
per batch. Biases bq/bk applied on device (ACT bias); bv/bo folded into a
constant host-side row (sum_k probs == 1).

On-device dataflow (all matmuls bf16 with fp32 PSUM accumulation):
  - host passes xT [4096, 2048] bf16 so projections produce q/k in
    [head_dim, tok] layout directly (no transposes anywhere).
  - RoPE with de-interleaved head_dim (host permutes wq/wk rows per head:
    evens then odds); rotation = swap 64-partition halves via SBUF->SBUF
    DMA; cos/sin staged as [128, 2048] with sign folded into sin.
  - phase 2 processes HEAD PAIRS sharing one kv head: the two heads'
    S_T / exp / PV interleave so each exp's latency hides behind the
    partner head's matmuls, and K/V weight loads are shared.
  - scores computed transposed S_T[k, q]; softmax denominators for both
    heads accumulate into ONE PSUM bank (partitions 0 / 32) via grouped
    ones-matmul bursts emitted after each 4-kt block (no exp-wait).
  - causal mask = binary multiply on the 4 diagonal-block offsets only;
    blocks fully above the diagonal are skipped.
  - PV output lands in [head_dim, tok] layout == lhsT of the wo matmul.
  - phase 3 keeps a 4-wide oc quad of wo resident so each attn lhsT load
    feeds 4 consecutive matmuls; wo streams in on the idle SP ring
    during phase 2.
"""
import sys

for _p in ("/opt/trn_rl_repo",):
    if _p not in sys.path:
        sys.path.insert(0, _p)

from contextlib import ExitStack

import ml_dtypes
import numpy as np

import concourse.bass as bass  # noqa: F401  (AP types used implicitly)
import concourse.tile as tile
from concourse import bacc, mybir
from concourse import masks as masks_mod
from concourse.bass_utils import run_bass_kernel_spmd

BF16 = ml_dtypes.bfloat16
F32 = np.float32

DIM = 4096
SEQ = 2048
HD = 128
N_HEADS = 32
N_KV = 8
QH = 8          # q heads per core
KVH = 2         # kv heads per core
NM = KVH + KVH + QH   # 12 m-tiles per core: V0 V1 K0 K1 Q0..Q7
TC = 512        # token chunk (PSUM free dim)
NKK = DIM // 128      # 32 contraction chunks
NTCH = SEQ // TC      # 4 token chunks
NTT = SEQ // 128      # 16 token tiles
SCALE = 1.0 / float(np.sqrt(HD))

# de-interleave permutation within one head_dim: evens then odds
PERM = np.concatenate([np.arange(0, HD, 2), np.arange(1, HD, 2)])

_CACHE = {}
LAST_RESULT = None


def _emit(tc_ctx, nc, xT, wT, woT, cosd, sind, mbin, bqd, bkd, out, phases=3,
          attn_mode="full", depth=3, sgrp=4, out_bf16=False):
    dt = mybir.dt
    AF = mybir.ActivationFunctionType
    with ExitStack() as ctx:
        cpool = ctx.enter_context(tc_ctx.tile_pool(name="consts", bufs=1))
        xpool = ctx.enter_context(tc_ctx.tile_pool(name="xp", bufs=11))
        wpool = ctx.enter_context(tc_ctx.tile_pool(name="wp", bufs=11))
        wopool = ctx.enter_context(tc_ctx.tile_pool(name="wop", bufs=1))
        qkv = ctx.enter_context(tc_ctx.tile_pool(name="qkv", bufs=1))
        stage = ctx.enter_context(tc_ctx.tile_pool(name="stage", bufs=2))
        probs_p = ctx.enter_context(tc_ctx.tile_pool(name="probs", bufs=5))
        aupool = ctx.enter_context(tc_ctx.tile_pool(name="aup", bufs=2))
        rpool = ctx.enter_context(tc_ctx.tile_pool(name="rp", bufs=2))
        outp = ctx.enter_context(tc_ctx.tile_pool(name="outp", bufs=1))
        # PSUM: 6-bank shared ring (tag "a") + 2 banks for PV pairs /
        # phase-1 V transposes (tag "pv")
        ps_a = ctx.enter_context(tc_ctx.tile_pool(name="psa", bufs=6, space="PSUM"))
        ps_sum = ctx.enter_context(tc_ctx.tile_pool(name="pssum", bufs=2, space="PSUM"))

        # ---- constant tiles (DMAs emitted after the first x/w loads so the
        # first matmuls aren't queued behind 4.5 MiB of constants) ----
        cos_sb = cpool.tile([HD, SEQ], dt.bfloat16, name="cos_sb")
        sin_sb = cpool.tile([HD, SEQ], dt.bfloat16, name="sin_sb")
        mask_sb = cpool.tile([HD, 4 * TC], dt.bfloat16, name="mask_sb")
        bq_sb = cpool.tile([HD, QH], dt.float32, name="bq_sb")
        bk_sb = cpool.tile([HD, KVH], dt.float32, name="bk_sb")
        ones_col = cpool.tile([128, 1], dt.bfloat16, name="ones_col")
        ident = cpool.tile([128, 128], dt.bfloat16, name="ident")

        def emit_const_loads():
            nc.sync.dma_start(cos_sb[:], cosd[:])
            nc.sync.dma_start(sin_sb[:], sind[:])
            nc.sync.dma_start(mask_sb[:], mbin[:])
            nc.sync.dma_start(bq_sb[:], bqd[:])
            nc.sync.dma_start(bk_sb[:], bkd[:])
            nc.vector.memset(ones_col[:], 1.0)
            masks_mod.make_identity(nc, ident[:])

        # ---- persistent per-core tensors ----
        q_roped = [qkv.tile([HD, SEQ], dt.bfloat16, name=f"qr{h}", tag=f"qr{h}")
                   for h in range(QH)]
        k_roped = [qkv.tile([HD, SEQ], dt.bfloat16, name=f"kr{g}", tag=f"kr{g}")
                   for g in range(KVH)]
        v_sb = [qkv.tile([128, KVH * HD], dt.bfloat16, name=f"vs{t}", tag=f"vs{t}")
                for t in range(NTT)]
        attn_out = [qkv.tile([HD, SEQ], dt.bfloat16, name=f"ao{h}", tag=f"ao{h}")
                    for h in range(QH)]

        # rope is emitted in three batched stages so no engine's in-order
        # queue chains one rope's tail into the next rope's PSUM release:
        #   stage 1 (ACT/DVE alternating): bias-add, PSUM -> SBUF, frees bank
        #   stage 2 (Pool): 64-partition half swap via SBUF->SBUF DMA
        #   stage 3 (DVE): cos/sin muls + add into the roped destination
        def rope_s1(pst, bias_ap, j, on_act):
            pre = stage.tile([128, TC], dt.bfloat16, name=f"pre{j}",
                             tag=f"pre{j}", bufs=1)
            if on_act:
                nc.scalar.activation(pre[:], pst[:], AF.Identity, bias=bias_ap)
            else:
                nc.vector.tensor_scalar_add(pre[:], pst[:], bias_ap)
            return pre

        def rope_s2(pre, j):
            rot = stage.tile([128, TC], dt.bfloat16, name=f"rot{j}",
                             tag=f"rot{j}", bufs=1)
            nc.gpsimd.dma_start(rot[0:64, :], pre[64:128, :])
            nc.gpsimd.dma_start(rot[64:128, :], pre[0:64, :])
            return rot

        def rope_s3(pre, rot, dst, t):
            sl = slice(t * TC, (t + 1) * TC)
            nc.vector.tensor_mul(pre[:], pre[:], cos_sb[:, sl])
            nc.vector.tensor_mul(rot[:], rot[:], sin_sb[:, sl])
            nc.vector.tensor_add(dst[:, sl], pre[:], rot[:])

        # ---- phase 1 (kk-outer streaming): per tok chunk, two m-groups of
        # 6 PSUM accumulators; each x / weight tile is consumed by 6 back-to-
        # back matmuls and freed, so DMA streams continuously. The V
        # transposes for a chunk are emitted AFTER group 1's matmuls so the
        # PE never waits on the group-0 epilogue drain.
        GRP = 6
        for t in range(NTCH):
            vtmp = [None, None]
            for grp in range(2):
                mb = grp * GRP
                pss = [ps_a.tile([128, TC], dt.float32, name=f"pj{j}", tag="a")
                       for j in range(GRP)]
                for kk in range(NKK):
                    # alternate the two HWDGE rings (sync / scalar) so x and
                    # weight streams run on parallel queues
                    ex = nc.sync if kk % 2 == 0 else nc.scalar
                    ew = nc.scalar if kk % 2 == 0 else nc.sync
                    xt = xpool.tile([128, TC], dt.bfloat16, name="xt", tag="xt")
                    ex.dma_start(
                        xt[:], xT[kk * 128:(kk + 1) * 128, t * TC:(t + 1) * TC])
                    wk6 = wpool.tile([128, GRP * 128], dt.bfloat16, name="wk6",
                                     tag="wk6")
                    ew.dma_start(
                        wk6[:],
                        wT[kk * 128:(kk + 1) * 128, mb * 128:(mb + GRP) * 128])
                    if t == 0 and grp == 0 and kk == 0:
                        emit_const_loads()

                    for j in range(GRP):
                        nc.tensor.matmul(pss[j][:],
                                         wk6[:, j * 128:(j + 1) * 128], xt[:],
                                         start=(kk == 0), stop=(kk == NKK - 1),
                                         skip_group_check=True)

                # group epilogue, batched by stage. Stage 1 is emitted at
                # high priority so the scheduler keeps it ahead of the next
                # group's DMA triggers on the ACT/DVE queues (the PSUM ring
                # release gates the next group's matmuls).
                ropes = []
                with tc_ctx.high_priority():
                    for j in range(GRP):
                        m = mb + j
                        on_act = (j % 2 == 0)
                        if m < KVH:  # V heads: keep [d, tok], no rope
                            vt_g = stage.tile([128, TC], dt.bfloat16,
                                              name=f"vtmp{m}", tag=f"vd{m}")
                            if on_act:
                                nc.scalar.copy(vt_g[:], pss[j][:])
                            else:
                                nc.vector.tensor_copy(vt_g[:], pss[j][:])
                            vtmp[m] = vt_g
                        elif m < 2 * KVH:  # K heads
                            g = m - KVH
                            pre = rope_s1(pss[j], bk_sb[:, g:g + 1], j, on_act)
                            ropes.append((pre, k_roped[g]))
                        else:  # Q heads
                            h = m - 2 * KVH
                            pre = rope_s1(pss[j], bq_sb[:, h:h + 1], j, on_act)
                            ropes.append((pre, q_roped[h]))
                rots = [rope_s2(pre, j) for j, (pre, _) in enumerate(ropes)]
                for (pre, dst), rot in zip(ropes, rots):
                    rope_s3(pre, rot, dst, t)
            # V transpose to [tok, d]; emitted after BOTH groups so the PE
            # pipeline isn't blocked on the group-0 epilogue. PSUM from the
            # "pv" ring (unused in phase 1).
            for i4 in range(4):
                tt = t * 4 + i4
                for g in range(KVH):
                    pvt = ps_sum.tile([128, 128], dt.bfloat16,
                                      name="pvt", tag="ps2")
                    nc.tensor.transpose(
                        pvt[:], vtmp[g][:, i4 * 128:(i4 + 1) * 128],
                        ident[:])
                    if (i4 + g) % 2 == 0:
                        nc.scalar.copy(v_sb[tt][:, g * 128:(g + 1) * 128],
                                       pvt[:])
                    else:
                        nc.vector.tensor_copy(
                            v_sb[tt][:, g * 128:(g + 1) * 128], pvt[:])

        if phases <= 1:
            # pin phase-1 outputs so DCE keeps the work
            for h in range(QH):
                src = q_roped[h][:, 0:256]
                nc.sync.dma_start(out[h * 128:(h + 1) * 128, 0:256] if out_bf16
                                  else out[h * 128:(h + 1) * 128, 0:128],
                                  src if out_bf16 else src.bitcast(dt.float32))
            for g in range(KVH):
                src = k_roped[g][:, 0:256]
                nc.sync.dma_start(
                    out[1024 + g * 128:1024 + (g + 1) * 128, 0:256] if out_bf16
                    else out[1024 + g * 128:1024 + (g + 1) * 128, 0:128],
                    src if out_bf16 else src.bitcast(dt.float32))
            for tt in range(NTT):
                nc.sync.dma_start(out[tt * 128:(tt + 1) * 128, 512:768] if out_bf16
                                  else out[tt * 128:(tt + 1) * 128, 512:640],
                                  v_sb[tt][:, :] if out_bf16
                                  else v_sb[tt][:, :].bitcast(dt.float32))
            return

        # ---- phase 2: attention per head (v1 structure: depth-3 ST
        # lookahead in the 6-slot ring, one spare slot so ST allocation
        # never serializes on exp completion) ----
        for h in range(QH):
            g = h // (QH // KVH)
            for qc in range(NTCH):
                nkt = 4 * qc + 4

                def st_of(kt, _h=h, _g=g, _qc=qc):
                    o = (kt - 4 * _qc) * 128 if kt >= 4 * _qc else 0
                    st = ps_a.tile([128, TC], dt.float32, name="st", tag="a")
                    nc.tensor.matmul(
                        st[:, o:TC],
                        k_roped[_g][:, kt * 128:(kt + 1) * 128],
                        q_roped[_h][:, _qc * TC + o:(_qc + 1) * TC],
                        start=True, stop=True)
                    return st, o

                sums = ps_sum.tile([1, TC], dt.float32, name="sums",
                                   tag="ps2")
                pv = ps_a.tile([128, TC], dt.float32, name="pv", tag="a")
                sts = [st_of(k) for k in range(min(depth, nkt))]
                for kt in range(nkt):
                    if kt + depth < nkt:
                        sts.append(st_of(kt + depth))
                    st, o = sts[kt]
                    e = probs_p.tile([128, TC], dt.bfloat16, name="expS",
                                     tag="e0")
                    nc.scalar.activation(e[:, o:TC], st[:, o:TC], AF.Exp,
                                         scale=SCALE)
                    if attn_mode != "nomask" and kt >= 4 * qc:
                        oi = kt - 4 * qc
                        nc.vector.tensor_mul(
                            e[:, o:TC], e[:, o:TC],
                            mask_sb[:, oi * TC + o:(oi + 1) * TC])
                    if attn_mode not in ("nosums", "nonorm"):
                        nc.tensor.matmul(sums[0:1, o:TC], ones_col[:],
                                         e[:, o:TC],
                                         start=(kt == 0), stop=(kt == nkt - 1),
                                         skip_group_check=True)
                    nc.tensor.matmul(pv[:, o:TC],
                                     v_sb[kt][:, g * HD:(g + 1) * HD],
                                     e[:, o:TC], start=(kt == 0),
                                     stop=(kt == nkt - 1),
                                     skip_group_check=True)
                if attn_mode in ("nosums", "nonorm"):
                    nc.vector.tensor_copy(
                        attn_out[h][:, qc * TC:(qc + 1) * TC], pv[:])
                    continue
                recip = rpool.tile([1, TC], dt.float32, name="recip",
                                   tag="r0")
                nc.vector.reciprocal(recip[:], sums[0:1, :])
                bcs = aupool.tile([128, TC], dt.float32, name="bcs", tag="b0")
                nc.gpsimd.partition_broadcast(bcs[:], recip[:])
                nc.vector.tensor_mul(attn_out[h][:, qc * TC:(qc + 1) * TC],
                                     pv[:], bcs[:])

        if phases <= 2:
            for h in range(QH):
                src = attn_out[h][:, 0:2048]
                nc.sync.dma_start(out[h * 128:(h + 1) * 128, 0:2048] if out_bf16
                                  else out[h * 128:(h + 1) * 128, 2048:3072],
                                  src if out_bf16 else src.bitcast(dt.float32))
            return

        # ---- phase 3: output projection (partial over this core's heads).
        # 4-wide oc quads of wo stay resident (loaded on the SP ring during
        # phase 2); each attn lhsT load feeds 4 consecutive matmuls.
        NQ = 4
        for q4 in range(DIM // TC // NQ):
            wos = []
            for h2 in range(QH):
                row = []
                for oc2 in range(NQ):
                    wo_t = wopool.tile([128, TC], dt.bfloat16,
                                       name=f"wo{h2}_{oc2}",
                                       tag=f"wo{h2}_{oc2}")
                    oc = q4 * NQ + oc2
                    nc.sync.dma_start(
                        wo_t[:],
                        woT[h2 * 128:(h2 + 1) * 128, oc * TC:(oc + 1) * TC])
                    row.append(wo_t)
                wos.append(row)
            for tt in range(NTT):
                pos = [ps_a.tile([128, TC], dt.float32, name=f"po{oc2}",
                                 tag="a") for oc2 in range(NQ)]
                for h2 in range(QH):
                    at = attn_out[h2][:, tt * 128:(tt + 1) * 128]
                    for oc2 in range(NQ):
                        nc.tensor.matmul(pos[oc2][:], at, wos[h2][oc2][:],
                                         start=(h2 == 0), stop=(h2 == QH - 1),
                                         skip_group_check=True)
                for oc2 in range(NQ):
                    oc = q4 * NQ + oc2
                    ob = outp.tile([128, TC],
                                   dt.bfloat16 if out_bf16 else dt.float32,
                                   name="ob", tag="ob", bufs=3)
                    if oc2 % 2 == 0:
                        nc.vector.tensor_copy(ob[:], pos[oc2][:])
                    else:
                        nc.scalar.copy(ob[:], pos[oc2][:])
                    nc.sync.dma_start(
                        out[tt * 128:(tt + 1) * 128, oc * TC:(oc + 1) * TC],
                        ob[:])


def build_nc(num_devices=8, reps=1, phases=3, attn_mode="full", depth=3,
             sgrp=4, out_bf16=False):
    nc = bacc.Bacc("TRN2", target_bir_lowering=False, debug=False,
                   enable_asserts=False, num_devices=num_devices)
    dt = mybir.dt
    xT = nc.dram_tensor("xT", [DIM, SEQ], dt.bfloat16, kind="ExternalInput").ap()
    wT = nc.dram_tensor("wT", [DIM, NM * 128], dt.bfloat16,
                        kind="ExternalInput").ap()
    woT = nc.dram_tensor("woT", [QH * HD, DIM], dt.bfloat16,
                         kind="ExternalInput").ap()
    cosd = nc.dram_tensor("cos128", [HD, SEQ], dt.bfloat16,
                          kind="ExternalInput").ap()
    sind = nc.dram_tensor("sin128s", [HD, SEQ], dt.bfloat16,
                          kind="ExternalInput").ap()
    mbin = nc.dram_tensor("maskbin", [HD, 4 * TC], dt.bfloat16,
                          kind="ExternalInput").ap()
    bqd = nc.dram_tensor("bq_sb", [HD, QH], dt.float32,
                         kind="ExternalInput").ap()
    bkd = nc.dram_tensor("bk_sb", [HD, KVH], dt.float32,
                         kind="ExternalInput").ap()
    out = nc.dram_tensor("out", [SEQ, DIM],
                         dt.bfloat16 if out_bf16 else dt.float32,
                         kind="ExternalOutput").ap()
    kw = dict(phases=phases, attn_mode=attn_mode, depth=depth, sgrp=sgrp,
              out_bf16=out_bf16)
    with tile.TileContext(nc) as tctx:
        if reps == 1:
            _emit(tctx, nc, xT, wT, woT, cosd, sind, mbin, bqd, bkd, out, **kw)
        else:
            with tctx.For_i(0, reps, 1):
                _emit(tctx, nc, xT, wT, woT, cosd, sind, mbin, bqd, bkd, out,
                      **kw)
    nc.compile()
    return nc


def _get_nc():
    if "nc" not in _CACHE:
        # bf16 output halves the 32 MiB/core result writeback; the host
        # accumulates the four TP partials in fp32
        _CACHE["nc"] = build_nc(out_bf16=True)
    return _CACHE["nc"]


def make_in_maps(x, freqs_cos, freqs_sin, mask, wq, bq, wk, bk, wv, bv):
    x = np.asarray(x, F32)
    xT_b = [np.ascontiguousarray(x[b].T).astype(BF16) for b in range(x.shape[0])]
    cosT = np.asarray(freqs_cos, F32).T  # [64, 2048]
    sinT = np.asarray(freqs_sin, F32).T
    cos128 = np.ascontiguousarray(np.vstack([cosT, cosT])).astype(BF16)
    sin128s = np.ascontiguousarray(np.vstack([-sinT, sinT])).astype(BF16)
    mask = np.asarray(mask, F32)
    mbin = np.zeros((HD, 4 * TC), F32)
    for oi in range(4):
        blk = mask[3 * TC:4 * TC, (12 + oi) * 128:(13 + oi) * 128]  # [512 q, 128 k]
        mbin[:, oi * TC:(oi + 1) * TC] = (blk == 0).T
    mbin = np.ascontiguousarray(mbin).astype(BF16)

    wq, wk, wv = (np.asarray(a, F32) for a in (wq, wk, wv))
    bq, bk = np.asarray(bq, F32), np.asarray(bk, F32)
    group_maps = []
    for g in range(4):
        wq_g = wq[g * 1024:(g + 1) * 1024].reshape(QH, HD, DIM)[:, PERM, :]
        wk_g = wk[g * 256:(g + 1) * 256].reshape(KVH, HD, DIM)[:, PERM, :]
        wv_g = wv[g * 256:(g + 1) * 256]
        wcat = np.concatenate(
            [wv_g, wk_g.reshape(KVH * HD, DIM), wq_g.reshape(QH * HD, DIM)], axis=0)
        wT_g = np.ascontiguousarray(wcat.T).astype(BF16)  # [4096, 1536]
        bq_g = np.ascontiguousarray(
            bq[g * 1024:(g + 1) * 1024].reshape(QH, HD)[:, PERM].T).astype(F32)
        bk_g = np.ascontiguousarray(
            bk[g * 256:(g + 1) * 256].reshape(KVH, HD)[:, PERM].T).astype(F32)
        group_maps.append(dict(wT=wT_g, bq_sb=bq_g, bk_sb=bk_g))

    in_maps = []
    for c in range(8):
        b, g = c // 4, c % 4
        in_maps.append(dict(xT=xT_b[b], cos128=cos128, sin128s=sin128s,
                            maskbin=mbin, **group_maps[g]))
    return in_maps


def kernel(x, freqs_cos, freqs_sin, mask, wq, bq, wk, bk, wv, bv, wo, bo):
    global LAST_RESULT
    nc = _get_nc()
    wo = np.asarray(wo, F32)
    in_maps = make_in_maps(x, freqs_cos, freqs_sin, mask, wq, bq, wk, bk, wv, bv)
    for c in range(8):
        g = c % 4
        in_maps[c]["woT"] = np.ascontiguousarray(
            wo[:, g * 1024:(g + 1) * 1024].T).astype(BF16)
    res = run_bass_kernel_spmd(nc, in_maps, core_ids=list(range(8)))
    LAST_RESULT = res
    outp = np.zeros((2, SEQ, DIM), F32)
    for c in range(8):
        outp[c // 4] += np.asarray(res.results[c]["out"], F32)
    bv = np.asarray(bv, F32)
    bo = np.asarray(bo, F32)
    bv_exp = np.broadcast_to(
        bv.reshape(N_KV, 1, HD), (N_KV, N_HEADS // N_KV, HD)).reshape(DIM)
    outp += (bv_exp @ wo.T + bo)[None, None, :].astype(F32)
    return outp
